# revision 30
# baseline (speedup 1.0000x reference)
"""Trainium2 Bass kernel for nn_DenseGNOBlock (B=4, N=8192, C=64).

Reference computes, per batch b:
    q = x Wq^T + bq ; k = x Wk^T + bk ; v = x Wv^T + bv
    kernel = q k^T / sqrt(C) ; integral = kernel v / N
    out = gelu(x Ww^T + bw + integral)

No softmax, so the N x N kernel reassociates away completely. With
augmented rows a_n = [1; x_n] and Gt = sum_n a_n a_n^T (65 x 65):
    out_n = gelu(Mt^T a_n),  Mt = Wtw^T + U Gt Wtv^T,  U = a Wtq^T Wtk
where Wt* = [b* | W*] and a = 1/(sqrt(C) N).

Layout/engineering:
- Gt accumulates directly in ONE PSUM bank: the host packs x as [1|x]
  chunk PAIRS in fp8e4m3 and the PE runs DoubleRow matmuls (K=256 per
  instruction, 0.5 cycles/row) -- 32 matmuls for the whole batch.
  fp8 in the Gram only perturbs the final output by ~1e-5 rel (the
  Gt-dependent term is alpha-scaled); the out-phase stays bf16.
- The output pass uses host-pretransposed bf16 x^T tiles (no PE
  transposes, no PSUM->SBUF copies): po = A @ Mt, gelu straight from
  PSUM. Folded weights ride as extra columns of the x^T tensor.
- Three DMA rings: SP and Pool (SWDGE) carry the latency-critical
  inputs (the ACT ring opens 1.3us late behind the Gelu table load, so
  it only gets the late x^T tiles). Gelu runs in two groups (9|23
  tiles) sized so the ACT engine never stalls; group stores overlap
  the next gelu (Pool), and the final store is split between the ACT
  ring (same-engine, no semaphore hop) and SP.

Sharding: 8 cores, core c -> batch b = c//2, half h = c%2. Each core
reads the full x_b (Gram needs all rows), writes its own half.
"""

import sys

for _p in ("/opt/trn_rl_repo", "/root/.axon_site/_ro/trn_rl_repo"):
    if _p not in sys.path:
        sys.path.append(_p)

import numpy as np
from contextlib import ExitStack

import concourse.bass as bass
import concourse.bacc as bacc
import concourse.mybir as mybir
import concourse.tile as tile
from concourse.bass_utils import run_bass_kernel_spmd

FP = mybir.dt.float32
BF = mybir.dt.bfloat16
F8 = mybir.dt.float8e4
AF = mybir.ActivationFunctionType
DR = mybir.MatmulPerfMode.DoubleRow

B, N, C = 4, 8192, 64
P = 128              # partitions
W = C + 1            # augmented width (ones column first)
NPR = N // (2 * P)   # 32 DoubleRow chunk pairs per batch
WP = 80              # padded pair width: DoubleRow needs slot stride % 16 == 0
HTILE = 32           # own-half out tiles of 128 rows
NCORES = 8
ALPHA = 1.0 / (np.sqrt(np.float32(C)) * np.float32(N))

XTW = HTILE * P      # 4096 data cols in xt
WPF = C + W + C      # wpk cols: wtvT | utT | wtwT
G_OUT = (9, 23)      # gelu group sizes (tiles)


def build_nc(act: str = "gelu") -> bass.Bass:
    act_fn = {"gelu": AF.Gelu, "identity": AF.Identity, "copy": AF.Copy}[act]
    nc = bacc.Bacc("TRN2", target_bir_lowering=False, debug=False)

    xg_d = nc.declare_dram_parameter("xg", [P, NPR * 2 * WP], F8, isOutput=False)
    xt_d = nc.declare_dram_parameter("xt", [W, XTW], BF, isOutput=False)
    wpk_d = nc.declare_dram_parameter("wpk", [W, WPF], BF, isOutput=False)
    out_d = nc.declare_dram_parameter("out", [P, HTILE * C], BF, isOutput=True)

    with ExitStack() as ctx:
        tc = ctx.enter_context(tile.TileContext(nc))
        const = ctx.enter_context(tc.tile_pool(name="const", bufs=1))
        ps_g = ctx.enter_context(tc.tile_pool(name="ps_g", bufs=1, space="PSUM"))
        ps_c = ctx.enter_context(tc.tile_pool(name="ps_c", bufs=1, space="PSUM"))
        ps_o = ctx.enter_context(tc.tile_pool(name="ps_o", bufs=1, space="PSUM"))

        xg = const.tile([P, NPR, 2, WP], F8)
        xt = const.tile([W, XTW], BF)
        wpk = const.tile([W, WPF], BF)
        wtvT = wpk[:, 0:C]
        utT = wpk[:, C : C + W]
        wtwT = wpk[:, C + W : WPF]

        # input schedule (v1 cost model: per-ring DMA slice = max(500,
        # free_bytes * 0.386) on the ISSUING engine, completion = slice end
        # + ~1717 (SP) / +1883 (Pool)). ACT opens with the 1.3us gelu table
        # load, so it only carries the late xt tiles; Pool's SWDGE is a
        # free second input ring.
        xgr = xg_d[:].rearrange("p (b two k) -> p b two k", two=2, k=WP)
        nc.gpsimd.dma_start(out=xg[:, 16:24], in_=xgr[:, 16:24])
        nc.sync.dma_start(out=xg[:, 0:8], in_=xgr[:, 0:8])
        nc.gpsimd.dma_start(out=xg[:, 24:32], in_=xgr[:, 24:32])
        nc.sync.dma_start(out=xg[:, 8:16], in_=xgr[:, 8:16])
        nc.gpsimd.dma_start(out=wpk[:], in_=wpk_d[:])
        nc.sync.dma_start(out=xt[:, 0 : 12 * P], in_=xt_d[:, 0 : 12 * P])
        nc.gpsimd.dma_start(out=xt[:, 24 * P :], in_=xt_d[:, 24 * P :])
        nc.scalar.dma_start(out=xt[:, 12 * P : 24 * P], in_=xt_d[:, 12 * P : 24 * P])

        # PE warm-up: keeps the p-state clock running from t~0
        warm = const.tile([P, C], BF)
        nc.vector.memset(warm[:], 1.0)
        wps = ps_c.tile([C, C], FP, tag="warm", bufs=1)
        for _ in range(3):
            nc.tensor.matmul(wps[:], warm[:], warm[:])
        nc.vector.tensor_copy(warm[0:C, :], wps[:])  # keep the tile "read"

        # --- Gt accumulation: 32 DoubleRow matmuls, one PSUM bank ---------
        gt_ps = ps_g.tile([W, W], FP)
        order = [*range(0, 8), *range(16, 24), *range(8, 16), *range(24, 32)]
        for i, pr in enumerate(order):
            nc.tensor.matmul(
                gt_ps[:], xg[:, pr, :, 0:W], xg[:, pr, :, 0:W],
                start=(i == 0), stop=(i == NPR - 1), perf_mode=DR,
            )

        # --- chain: Mt = wtwT + U (Gt wtvT) ------------------------------
        gt_sb = const.tile([W, W], BF)
        nc.vector.tensor_copy(gt_sb[:], gt_ps[:])
        t1_ps = ps_c.tile([W, C], FP, tag="chain", bufs=1)
        nc.tensor.matmul(t1_ps[:], gt_sb[:], wtvT)   # Gt^T wtvT = Gt wtvT
        t1_sb = const.tile([W, C], BF)
        nc.vector.tensor_copy(t1_sb[:], t1_ps[:])
        macc_ps = ps_c.tile([W, C], FP, tag="chain", bufs=1)
        nc.tensor.matmul(macc_ps[:], utT, t1_sb[:])  # (U^T)^T T1 = U T1
        mt_sb = const.tile([W, C], BF)
        nc.vector.tensor_add(mt_sb[:], macc_ps[:], wtwT)

        # --- own half: out = gelu(A @ Mt) straight from PSUM -------------
        # stores ride the otherwise-idle Pool SWDGE ring (v1 model charges
        # the DMA slice to the issuing engine) except the last, which goes
        # wherever its completion (slice + 1717ns) lands earliest
        osb = const.tile([P, HTILE * C], BF)
        t0 = 0
        for q, ntile in enumerate(G_OUT):
            po = ps_o.tile([P, ntile, C], FP, tag=f"po{q}", bufs=1)
            for j in range(ntile):
                t = t0 + j
                nc.tensor.matmul(po[:, j, :], xt[:, t * P : (t + 1) * P], mt_sb[:])
            nc.scalar.activation(
                osb[:, t0 * C : (t0 + ntile) * C],
                po[:].rearrange("p a c -> p (a c)"),
                act_fn,
            )
            if q == 0:
                nc.gpsimd.dma_start(
                    out=out_d[:, t0 * C : (t0 + ntile) * C],
                    in_=osb[:, t0 * C : (t0 + ntile) * C],
                )
            else:
                # final group: split across the ACT ring (same-engine order,
                # no sem hop after the gelu) and SP
                cut = t0 + 13
                nc.scalar.dma_start(
                    out=out_d[:, t0 * C : cut * C],
                    in_=osb[:, t0 * C : cut * C],
                )
                nc.sync.dma_start(
                    out=out_d[:, cut * C : (t0 + ntile) * C],
                    in_=osb[:, cut * C : (t0 + ntile) * C],
                )
            t0 += ntile

    nc.compile()
    return nc


_NC_CACHE = None


def _get_nc() -> bass.Bass:
    global _NC_CACHE
    if _NC_CACHE is None:
        _NC_CACHE = build_nc()
    return _NC_CACHE


def make_in_maps(inputs: dict) -> list[dict]:
    import ml_dtypes

    x = np.asarray(inputs["x"], dtype=np.float32)
    Wq, Wk, Wv, Ww = (np.asarray(inputs[k], np.float32) for k in ("Wq", "Wk", "Wv", "Ww"))
    bq, bk, bv, bw = (np.asarray(inputs[k], np.float32) for k in ("bq", "bk", "bv", "bw"))

    def aug(Wm, bm):  # Wt* = [b* | W*]  [64, 65]
        return np.concatenate([bm[:, None], Wm], axis=1)

    wtq, wtk, wtv, wtw = aug(Wq, bq), aug(Wk, bk), aug(Wv, bv), aug(Ww, bw)
    U = (ALPHA * (wtq.T @ wtk)).astype(np.float32)          # [65, 65]

    in_maps = []
    for c in range(NCORES):
        b, h = c // 2, c % 2
        xb = x[b]                                   # [8192, 64]
        # pair pr, slot i, partition p -> row (2*pr+i)*128 + p
        xg = np.zeros((P, NPR, 2, WP), np.float32)
        xg[:, :, :, 0] = 1.0
        xg[:, :, :, 1 : 1 + C] = xb.reshape(NPR, 2, P, C).transpose(2, 0, 1, 3)

        own = xb[h * (N // 2) : (h + 1) * (N // 2)]  # [4096, 64]
        # xt column t*128+p  <->  own row p*32+t ; ones row first
        xt = np.empty((W, XTW), np.float32)
        xt[0] = 1.0
        xt[1:] = own.reshape(P, HTILE, C).transpose(2, 1, 0).reshape(C, XTW)
        wpk = np.empty((W, WPF), np.float32)
        wpk[:, 0:C] = wtv.T
        wpk[:, C : C + W] = U.T
        wpk[:, C + W :] = wtw.T
        in_maps.append(
            dict(
                xg=np.ascontiguousarray(
                    xg.reshape(P, NPR * 2 * WP).astype(ml_dtypes.float8_e4m3)
                ),
                xt=np.ascontiguousarray(xt.astype(ml_dtypes.bfloat16)),
                wpk=np.ascontiguousarray(wpk.astype(ml_dtypes.bfloat16)),
            )
        )
    return in_maps


def kernel(**inputs) -> np.ndarray:
    nc = _get_nc()
    in_maps = make_in_maps(inputs)
    res = run_bass_kernel_spmd(nc, in_maps, list(range(NCORES)))
    out = np.empty((B, N, C), np.float32)
    for c in range(NCORES):
        b, h = c // 2, c % 2
        oc = np.asarray(res.results[c]["out"]).astype(np.float32)
        # out[p, t*64:(t+1)*64] = own row p*32+t
        own = oc.reshape(P, HTILE, C).reshape(N // 2, C)
        out[b, h * (N // 2) : (h + 1) * (N // 2)] = own
    return out


# revision 34
# speedup vs baseline: 1.0121x; 1.0121x over previous
"""Trainium2 Bass kernel for nn_DenseGNOBlock (B=4, N=8192, C=64).

Reference computes, per batch b:
    q = x Wq^T + bq ; k = x Wk^T + bk ; v = x Wv^T + bv
    kernel = q k^T / sqrt(C) ; integral = kernel v / N
    out = gelu(x Ww^T + bw + integral)

No softmax, so the N x N kernel reassociates away completely. With
augmented rows a_n = [1; x_n] and Gt = sum_n a_n a_n^T (65 x 65):
    out_n = gelu(Mt^T a_n),  Mt = Wtw^T + U Gt Wtv^T,  U = a Wtq^T Wtk
where Wt* = [b* | W*] and a = 1/(sqrt(C) N).

Layout/engineering:
- Gt accumulates directly in ONE PSUM bank: the host packs x as [1|x]
  chunk PAIRS in fp8e4m3 and the PE runs DoubleRow matmuls (K=256 per
  instruction, 0.5 cycles/row) -- 32 matmuls for the whole batch.
  fp8 in the Gram only perturbs the final output by ~1e-5 rel (the
  Gt-dependent term is alpha-scaled); the out-phase stays bf16.
- The output pass uses host-pretransposed bf16 x^T tiles (no PE
  transposes, no PSUM->SBUF copies): po = A @ Mt, gelu straight from
  PSUM. Folded weights ride as extra columns of the x^T tensor.
- Three DMA rings: SP and Pool (SWDGE) carry the latency-critical
  inputs (the ACT ring opens 1.3us late behind the Gelu table load, so
  it only gets the late x^T tiles). Gelu runs in two groups (9|23
  tiles) sized so the ACT engine never stalls; group stores overlap
  the next gelu (Pool), and the final store is split between the ACT
  ring (same-engine, no semaphore hop) and SP.

Sharding: 8 cores, core c -> batch b = c//2, half h = c%2. Each core
reads the full x_b (Gram needs all rows), writes its own half.
"""

import sys

for _p in ("/opt/trn_rl_repo", "/root/.axon_site/_ro/trn_rl_repo"):
    if _p not in sys.path:
        sys.path.append(_p)

import numpy as np
from contextlib import ExitStack

import concourse.bass as bass
import concourse.bacc as bacc
import concourse.mybir as mybir
import concourse.tile as tile
from concourse.bass_utils import run_bass_kernel_spmd

FP = mybir.dt.float32
BF = mybir.dt.bfloat16
F8 = mybir.dt.float8e4
AF = mybir.ActivationFunctionType
DR = mybir.MatmulPerfMode.DoubleRow

B, N, C = 4, 8192, 64
P = 128              # partitions
W = C + 1            # augmented width (ones column first)
NPR = N // (2 * P)   # 32 DoubleRow chunk pairs per batch
WP = 80              # padded pair width: DoubleRow needs slot stride % 16 == 0
HTILE = 32           # own-half out tiles of 128 rows
NCORES = 8
ALPHA = 1.0 / (np.sqrt(np.float32(C)) * np.float32(N))

XTW = HTILE * P      # 4096 data cols in xt
WPF = C + W + C      # wpk cols: wtvT | utT | wtwT
G_OUT = (10, 22)     # gelu group sizes (tiles)


def build_nc(act: str = "gelu") -> bass.Bass:
    act_fn = {"gelu": AF.Gelu, "identity": AF.Identity, "copy": AF.Copy}[act]
    nc = bacc.Bacc("TRN2", target_bir_lowering=False, debug=False)

    xg_d = nc.declare_dram_parameter("xg", [P, NPR * 2 * WP], F8, isOutput=False)
    xt_d = nc.declare_dram_parameter("xt", [W, XTW], BF, isOutput=False)
    wpk_d = nc.declare_dram_parameter("wpk", [W, WPF], BF, isOutput=False)
    out_d = nc.declare_dram_parameter("out", [P, HTILE * C], BF, isOutput=True)

    with ExitStack() as ctx:
        tc = ctx.enter_context(tile.TileContext(nc))
        const = ctx.enter_context(tc.tile_pool(name="const", bufs=1))
        ps_g = ctx.enter_context(tc.tile_pool(name="ps_g", bufs=1, space="PSUM"))
        ps_c = ctx.enter_context(tc.tile_pool(name="ps_c", bufs=1, space="PSUM"))
        ps_o = ctx.enter_context(tc.tile_pool(name="ps_o", bufs=1, space="PSUM"))

        xg = const.tile([P, NPR, 2, WP], F8)
        xt = const.tile([W, XTW], BF)
        wpk = const.tile([W, WPF], BF)
        wtvT = wpk[:, 0:C]
        utT = wpk[:, C : C + W]
        wtwT = wpk[:, C + W : WPF]

        # input schedule (v1 cost model: per-ring DMA slice = max(500,
        # free_bytes * 0.386) on the ISSUING engine, completion = slice end
        # + ~1717 (SP) / +1883 (Pool)). ACT opens with the 1.3us gelu table
        # load, so it only carries the late xt tiles; Pool's SWDGE is a
        # free second input ring.
        xgr = xg_d[:].rearrange("p (b two k) -> p b two k", two=2, k=WP)
        nc.gpsimd.dma_start(out=xg[:, 16:24], in_=xgr[:, 16:24])
        nc.sync.dma_start(out=xg[:, 0:8], in_=xgr[:, 0:8])
        nc.gpsimd.dma_start(out=xg[:, 24:32], in_=xgr[:, 24:32])
        nc.sync.dma_start(out=xg[:, 8:16], in_=xgr[:, 8:16])
        nc.gpsimd.dma_start(out=wpk[:], in_=wpk_d[:])
        nc.sync.dma_start(out=xt[:, 0 : 12 * P], in_=xt_d[:, 0 : 12 * P])
        nc.gpsimd.dma_start(out=xt[:, 24 * P :], in_=xt_d[:, 24 * P :])
        nc.scalar.dma_start(out=xt[:, 12 * P : 24 * P], in_=xt_d[:, 12 * P : 24 * P])

        # PE warm-up: keeps the p-state clock running from t~0
        warm = const.tile([P, C], BF)
        nc.vector.memset(warm[:], 1.0)
        wps = ps_c.tile([C, C], FP, tag="warm", bufs=1)
        for _ in range(3):
            nc.tensor.matmul(wps[:], warm[:], warm[:])
        nc.vector.tensor_copy(warm[0:C, :], wps[:])  # keep the tile "read"

        # --- Gt accumulation: 32 DoubleRow matmuls, one PSUM bank ---------
        gt_ps = ps_g.tile([W, W], FP)
        order = [*range(0, 8), *range(16, 24), *range(8, 16), *range(24, 32)]
        for i, pr in enumerate(order):
            nc.tensor.matmul(
                gt_ps[:], xg[:, pr, :, 0:W], xg[:, pr, :, 0:W],
                start=(i == 0), stop=(i == NPR - 1), perf_mode=DR,
            )

        # --- chain: Mt = wtwT + U (Gt wtvT) ------------------------------
        gt_sb = const.tile([W, W], BF)
        nc.vector.tensor_copy(gt_sb[:], gt_ps[:])
        t1_ps = ps_c.tile([W, C], FP, tag="chain", bufs=1)
        nc.tensor.matmul(t1_ps[:], gt_sb[:], wtvT)   # Gt^T wtvT = Gt wtvT
        t1_sb = const.tile([W, C], BF)
        nc.vector.tensor_copy(t1_sb[:], t1_ps[:])
        macc_ps = ps_c.tile([W, C], FP, tag="chain", bufs=1)
        nc.tensor.matmul(macc_ps[:], utT, t1_sb[:])  # (U^T)^T T1 = U T1
        mt_sb = const.tile([W, C], BF)
        nc.vector.tensor_add(mt_sb[:], macc_ps[:], wtwT)

        # --- own half: out = gelu(A @ Mt) straight from PSUM -------------
        # stores ride the otherwise-idle Pool SWDGE ring (v1 model charges
        # the DMA slice to the issuing engine) except the last, which goes
        # wherever its completion (slice + 1717ns) lands earliest
        osb = const.tile([P, HTILE * C], BF)
        t0 = 0
        for q, ntile in enumerate(G_OUT):
            po = ps_o.tile([P, ntile, C], FP, tag=f"po{q}", bufs=1)
            for j in range(ntile):
                t = t0 + j
                nc.tensor.matmul(po[:, j, :], xt[:, t * P : (t + 1) * P], mt_sb[:])
            nc.scalar.activation(
                osb[:, t0 * C : (t0 + ntile) * C],
                po[:].rearrange("p a c -> p (a c)"),
                act_fn,
            )
            if q == 0:
                nc.gpsimd.dma_start(
                    out=out_d[:, t0 * C : (t0 + ntile) * C],
                    in_=osb[:, t0 * C : (t0 + ntile) * C],
                )
            else:
                # final group: split across the ACT ring (same-engine order,
                # no sem hop after the gelu) and SP
                cut = t0 + 12
                nc.scalar.dma_start(
                    out=out_d[:, t0 * C : cut * C],
                    in_=osb[:, t0 * C : cut * C],
                )
                nc.sync.dma_start(
                    out=out_d[:, cut * C : (t0 + ntile) * C],
                    in_=osb[:, cut * C : (t0 + ntile) * C],
                )
            t0 += ntile

    nc.compile()
    return nc


_NC_CACHE = None


def _get_nc() -> bass.Bass:
    global _NC_CACHE
    if _NC_CACHE is None:
        _NC_CACHE = build_nc()
    return _NC_CACHE


def make_in_maps(inputs: dict) -> list[dict]:
    import ml_dtypes

    x = np.asarray(inputs["x"], dtype=np.float32)
    Wq, Wk, Wv, Ww = (np.asarray(inputs[k], np.float32) for k in ("Wq", "Wk", "Wv", "Ww"))
    bq, bk, bv, bw = (np.asarray(inputs[k], np.float32) for k in ("bq", "bk", "bv", "bw"))

    def aug(Wm, bm):  # Wt* = [b* | W*]  [64, 65]
        return np.concatenate([bm[:, None], Wm], axis=1)

    wtq, wtk, wtv, wtw = aug(Wq, bq), aug(Wk, bk), aug(Wv, bv), aug(Ww, bw)
    U = (ALPHA * (wtq.T @ wtk)).astype(np.float32)          # [65, 65]

    in_maps = []
    for c in range(NCORES):
        b, h = c // 2, c % 2
        xb = x[b]                                   # [8192, 64]
        # pair pr, slot i, partition p -> row (2*pr+i)*128 + p
        xg = np.zeros((P, NPR, 2, WP), np.float32)
        xg[:, :, :, 0] = 1.0
        xg[:, :, :, 1 : 1 + C] = xb.reshape(NPR, 2, P, C).transpose(2, 0, 1, 3)

        own = xb[h * (N // 2) : (h + 1) * (N // 2)]  # [4096, 64]
        # xt column t*128+p  <->  own row p*32+t ; ones row first
        xt = np.empty((W, XTW), np.float32)
        xt[0] = 1.0
        xt[1:] = own.reshape(P, HTILE, C).transpose(2, 1, 0).reshape(C, XTW)
        wpk = np.empty((W, WPF), np.float32)
        wpk[:, 0:C] = wtv.T
        wpk[:, C : C + W] = U.T
        wpk[:, C + W :] = wtw.T
        in_maps.append(
            dict(
                xg=np.ascontiguousarray(
                    xg.reshape(P, NPR * 2 * WP).astype(ml_dtypes.float8_e4m3)
                ),
                xt=np.ascontiguousarray(xt.astype(ml_dtypes.bfloat16)),
                wpk=np.ascontiguousarray(wpk.astype(ml_dtypes.bfloat16)),
            )
        )
    return in_maps


def kernel(**inputs) -> np.ndarray:
    nc = _get_nc()
    in_maps = make_in_maps(inputs)
    res = run_bass_kernel_spmd(nc, in_maps, list(range(NCORES)))
    out = np.empty((B, N, C), np.float32)
    for c in range(NCORES):
        b, h = c // 2, c % 2
        oc = np.asarray(res.results[c]["out"]).astype(np.float32)
        # out[p, t*64:(t+1)*64] = own row p*32+t
        own = oc.reshape(P, HTILE, C).reshape(N // 2, C)
        out[b, h * (N // 2) : (h + 1) * (N // 2)] = own
    return out
